# revision 3
# baseline (speedup 1.0000x reference)
"""EMA (ExponentialMovingAverage, adjust=True) over (32, 4096, 256) f32 on 8 trn2 cores.

Math: the reference recurrence is
    e_0 = x_0;  e_t = (alpha*x_t + oma*e_{t-1}) / w_t,  w_t = max(1-oma^(t+1), 1e-10)
i.e. e_t = a_t*e_{t-1} + b_t*x_t with a_t = oma/w_t, b_t = alpha/w_t. Coefficients
depend only on t, so the whole scan is E = L @ X with lower-triangular
L[t,s] = b_s * prod_{u=s+1..t} a_u. L decays like oma^(t-s) (oma=0.923), so it is
effectively banded: truncating at lag >= 129 changes the result by ~3e-7 rel.

Kernel: chunk time into C=128 blocks. For chunk k:
    E_k = O_k @ X_{k-1} + D_k @ X_k     (two matmuls, PSUM-accumulated)
where D_k is the in-chunk lower-tri block and O_k the previous-chunk block of L.
No carry chain, no serial dependency — every chunk is independent. w_t rounds to
1.0f for t >= 216, so D is shared for chunks >= 2 and O for chunks >= 3; only 6
distinct 128x128 matrices total, uploaded once.

Precision/traffic: the harness gate is rel_err < 2e-2; bf16 I/O (x, W, y all
bf16, PE accumulates in f32 PSUM) gives ~2.7e-3 rel err and HALVES the HBM
traffic vs f32: 8 MiB in + 8 MiB out per core, against a ~358 GB/s per-core
HBM limit -> ~47 us floor (the f32 baseline's floor was ~104 us).

Sharding: pure data parallelism — 4 of the 32 batches per core, no comms.
Host converts f32->bf16 on the way in and bf16->f32 on the way out.

Schedule per core: loads grouped 8 chunks (512 KiB) per DMA on the SP HWDGE
ring, stores on the ACT ring; PSUM->SBUF downcast copies alternate between the
ACT and DVE engines so neither becomes the bottleneck.
"""

import os
import sys

import numpy as np

for _p in ("/opt/trn_rl_repo",):
    if os.path.isdir(_p) and _p not in sys.path:
        sys.path.append(_p)

import ml_dtypes

import concourse.bass as bass
import concourse.mybir as mybir
from concourse.bass_utils import run_bass_kernel_spmd
from concourse.tile import TileContext
from concourse.vector_clock import ScopedClock

# ---------------------------------------------------------------------------
# Workaround: TileContext's tail drain puts every owed proc's sem wait on one
# Drain instruction; walrus codegen allows only one sync wait per instruction,
# so any kernel touching more than a few procs fails codegen with "Too many
# sync wait commands". Split the waits across SP nops, one wait each.
# ---------------------------------------------------------------------------
_MAX_WAITS = 1


def _split_drain_and_barrier(self, tick_clock, wait_clock):
    carrier = self.nc.sync.nop(nofuse=True, hint="drain_wait_carrier")
    wait_clock.add_sem_waits(
        carrier.ins, ScopedClock({None: tick_clock.global_clock})
    )
    si = carrier.ins.sync_info
    if si is not None and len(si.on_wait) > _MAX_WAITS:
        waits = list(si.on_wait)
        carrier.ins.sync_info = mybir.SyncInfo(
            on_wait=waits[:_MAX_WAITS], on_update=list(si.on_update)
        )
        rest = waits[_MAX_WAITS:]
        for i in range(0, len(rest), _MAX_WAITS):
            nop = self.nc.sync.nop(nofuse=True, hint="drain_wait_spill")
            nop.ins.sync_info = mybir.SyncInfo(
                on_wait=rest[i : i + _MAX_WAITS], on_update=[]
            )
    self.nc.sync.drain()

    self.nc.all_engine_barrier()
    assert self.sems is not None
    popped = self.nc._tile_sem_poison_stack.pop()
    assert popped is self._sem_poison
    self.nc.clear_and_free_semaphores(list(self.sems.allocated().values()))
    self.nc.all_engine_barrier()


TileContext._drain_and_barrier = _split_drain_and_barrier

# ---------------------------------------------------------------------------
# Same walrus limitation for regular instructions: Tile attaches up to ~4 sem
# waits to one instruction; this walrus rejects more than WAIT_CAPS[type] sync
# wait commands per instruction. Spill the extras onto same-engine NoOps
# inserted right before the instruction (engines execute their stream in BB
# order, so the waits still complete before the instruction runs).
# ---------------------------------------------------------------------------

_WAIT_CAP_DEFAULT = 1
_WAIT_CAPS = {
    "InstEventSemaphore": 2,
}
_spill_counter = [0]


def spill_excess_waits(nc):
    for fn in nc.m.functions:
        for bb in fn.blocks:
            insts = bb.instructions
            i = 0
            while i < len(insts):
                inst = insts[i]
                si = inst.sync_info
                if si is None or not si.on_wait:
                    i += 1
                    continue
                cap = _WAIT_CAPS.get(type(inst).__name__, _WAIT_CAP_DEFAULT)
                waits = list(si.on_wait)
                if len(waits) <= cap:
                    i += 1
                    continue
                keep = waits[-cap:]
                rest = waits[:-cap]
                inst.sync_info = mybir.SyncInfo(
                    on_wait=keep, on_update=list(si.on_update)
                )
                carriers = []
                for j in range(0, len(rest), _WAIT_CAP_DEFAULT):
                    _spill_counter[0] += 1
                    nop = mybir.InstNoOp(name=f"spillw-{_spill_counter[0]}")
                    nop.engine = inst.engine
                    nop.sync_info = mybir.SyncInfo(
                        on_wait=rest[j : j + _WAIT_CAP_DEFAULT], on_update=[]
                    )
                    carriers.append(nop)
                for off, nop in enumerate(carriers):
                    insts.insert(i + off, nop)
                i += len(carriers) + 1


B, T, F = 32, 4096, 256
NCORES = 8
BL = B // NCORES  # local batches per core
C = 128  # time chunk
NCHUNK = T // C
NMAT = 6  # D0, D1, Dc, O1, O2, Oc


def _coeffs():
    """The 6 distinct 128x128 L-blocks, as lhsT ([t_in, t_out]) bf16."""
    alpha = np.float64(np.float32(2.0 / 26.0))
    oma = np.float64(np.float32(1.0 - 2.0 / 26.0))
    t = np.arange(1, T, dtype=np.float32)
    w32 = np.maximum(
        np.float32(1.0) - np.float32(oma) ** (t + np.float32(1.0)),
        np.float32(1e-10),
    ).astype(np.float32)
    a = np.ones(T, dtype=np.float64)
    b = np.zeros(T, dtype=np.float64)
    a[1:] = oma / w32.astype(np.float64)
    b[1:] = alpha / w32.astype(np.float64)
    b[0] = 1.0

    def build_pair(k):
        lo = k * C
        base = max(0, lo - C)
        P = np.cumprod(a[base : lo + C])

        def L(tt, ss):
            return b[ss] * (P[tt - base] / P[ss - base])

        D = np.zeros((C, C))
        O = np.zeros((C, C))
        for tt in range(C):
            for ss in range(tt + 1):
                D[tt, ss] = L(lo + tt, lo + ss)
            if k > 0:
                for ss in range(C):
                    O[tt, ss] = L(lo + tt, lo - C + ss)
        return D, O

    d0, _ = build_pair(0)
    d1, o1 = build_pair(1)
    dc, o2 = build_pair(2)
    _, oc = build_pair(3)
    # lhsT layout per matrix: [t_in (partition), t_out]; stack -> (128, 6, 128)
    wt = np.stack(
        [m.T for m in (d0, d1, dc, o1, o2, oc)], axis=0
    ).astype(np.float32)
    wt = np.ascontiguousarray(wt.transpose(1, 0, 2)).astype(ml_dtypes.bfloat16)
    return wt


_WT = _coeffs()
# matrix indices in _WT
D0, D1, DC, O1, O2, OC = range(6)


def _sel(k):
    """(off_idx or None, diag_idx) for chunk k."""
    if k == 0:
        return None, D0
    if k == 1:
        return O1, D1
    if k == 2:
        return O2, DC
    return OC, DC


GROUP = 8  # chunks per DMA group (512 KiB bf16 loads)


def build_nc(repeats=1, variant="full", xbufs=8, ebufs=8, spill=True,
             bench_io=False, group=GROUP):
    f32 = mybir.dt.float32
    bf16 = mybir.dt.bfloat16
    nc = bass.Bass(trn_type="TRN2")
    if bench_io:
        # Timing-only NEFF: tiny external I/O (dispatch payload over axon is
        # per-call, ~100ms for the real 384MB), real traffic hits internal
        # DRAM scratch instead. Data is garbage; timing is identical.
        xin = nc.dram_tensor("x", [1, 4], bf16, kind="ExternalInput")
        wt = nc.dram_tensor("wt", [128, NMAT, C], bf16, kind="ExternalInput")
        yout = nc.dram_tensor("y", [1, 4], bf16, kind="ExternalOutput")
        x = nc.dram_tensor("xscratch", [BL, T, F], bf16)
        y = nc.dram_tensor("yscratch", [BL, T, F], bf16)
    else:
        x = nc.dram_tensor("x", [BL, T, F], bf16, kind="ExternalInput")
        wt = nc.dram_tensor("wt", [128, NMAT, C], bf16, kind="ExternalInput")
        y = nc.dram_tensor("y", [BL, T, F], bf16, kind="ExternalOutput")

    with TileContext(nc) as tc:
        with (
            tc.tile_pool(name="wpool", bufs=1) as wpool,
            tc.tile_pool(name="xpool", bufs=xbufs) as xpool,
            tc.tile_pool(name="epool", bufs=ebufs) as epool,
            tc.tile_pool(name="psum", bufs=8, space="PSUM") as ppool,
        ):
            w_tile = wpool.tile([128, NMAT, C], bf16)
            nc.sync.dma_start(out=w_tile[:], in_=wt[:])
            if bench_io:
                iot = wpool.tile([1, 4], bf16, name="iot")
                nc.sync.dma_start(out=iot[:], in_=xin[:])
                nc.sync.dma_start(out=yout[:], in_=iot[:])
            for _rep in range(repeats):
                _emit_pass(nc, tc, x, y, w_tile,
                           (xpool, epool, ppool), variant, group=group)
    if spill:
        spill_excess_waits(nc)
    return nc


def _emit_pass(nc, tc, x, y, w_tile, pools, variant="full", group=GROUP):
    xpool, epool, ppool = pools
    f32 = mybir.dt.float32
    bf16 = mybir.dt.bfloat16

    xr = x.rearrange("b (g t) f -> b t g f", t=C)
    yr = y.rearrange("b (g t) f -> b t g f", t=C)

    if variant.startswith("dmabig"):
        # pure-DMA bandwidth probe with `nch` chunks per DMA
        nch = int(variant[6:])
        for g0 in range(0, NCHUNK, nch):
            for b in range(BL):
                xt = xpool.tile([C, nch, F], bf16, tag="xtb")
                nc.sync.dma_start(out=xt[:], in_=xr[b, :, g0 : g0 + nch, :])
                nc.scalar.dma_start(out=yr[b, :, g0 : g0 + nch, :], in_=xt[:])
        return

    prev = [None] * BL  # (tile, j) holding chunk k-1 per batch
    for g0 in range(0, NCHUNK, group):
        xts, ets = [], []
        for b in range(BL):
            xt = xpool.tile([C, group, F], bf16, tag="xt")
            nc.sync.dma_start(out=xt[:], in_=xr[b, :, g0 : g0 + group, :])
            xts.append(xt)
            ets.append(
                epool.tile([C, group, F], bf16, tag="et", name=f"et_{g0}_{b}")
            )
        for j in range(group):
            k = g0 + j
            osel, dsel = _sel(k)
            for b in range(BL):
                pt = ppool.tile([C, F], f32, tag="pt")
                if osel is None:
                    nc.tensor.matmul(
                        pt[:], w_tile[:, dsel, :], xts[b][:, j, :],
                        start=True, stop=True,
                    )
                else:
                    pxt, pj = (xts[b], j - 1) if j > 0 else prev[b]
                    nc.tensor.matmul(
                        pt[:], w_tile[:, osel, :], pxt[:, pj, :],
                        start=True, stop=False,
                    )
                    nc.tensor.matmul(
                        pt[:], w_tile[:, dsel, :], xts[b][:, j, :],
                        start=False, stop=True,
                    )
                # PSUM f32 -> SBUF bf16 downcast. ALL copies on ACT, and the
                # store dma_start is also issued from ACT: same-engine FIFO
                # order makes copy-before-store structural (no cross-engine
                # sem wait on the store path — the one spot a dropped/raced
                # sem would silently zero a whole output block).
                nc.scalar.copy(out=ets[b][:, j, :], in_=pt[:])
        for b in range(BL):
            prev[b] = (xts[b], group - 1)
            nc.scalar.dma_start(
                out=yr[b, :, g0 : g0 + group, :], in_=ets[b][:]
            )


_NC = None


def get_nc():
    global _NC
    if _NC is None:
        _NC = build_nc()
    return _NC


def kernel(x):
    x = np.asarray(x)
    assert x.shape == (B, T, F), x.shape
    xb = np.ascontiguousarray(x.astype(ml_dtypes.bfloat16))
    nc = get_nc()
    in_maps = [
        {"x": xb[c * BL : (c + 1) * BL], "wt": _WT} for c in range(NCORES)
    ]
    res = run_bass_kernel_spmd(nc, in_maps, core_ids=list(range(NCORES)))
    out = np.concatenate(
        [res.results[c]["y"].astype(np.float32) for c in range(NCORES)], axis=0
    )
    return out


# revision 4
# speedup vs baseline: 1.1350x; 1.1350x over previous
"""EMA (ExponentialMovingAverage, adjust=True) over (32, 4096, 256) f32 on 8 trn2 cores.

Math: the reference recurrence is
    e_0 = x_0;  e_t = (alpha*x_t + oma*e_{t-1}) / w_t,  w_t = max(1-oma^(t+1), 1e-10)
i.e. e_t = a_t*e_{t-1} + b_t*x_t with a_t = oma/w_t, b_t = alpha/w_t. Coefficients
depend only on t, so the whole scan is E = L @ X with lower-triangular
L[t,s] = b_s * prod_{u=s+1..t} a_u. L decays like oma^(t-s) (oma=0.923), so it is
effectively banded: truncating at lag >= 129 changes the result by ~3e-7 rel.

Kernel: chunk time into C=128 blocks. For chunk k:
    E_k = O_k @ X_{k-1} + D_k @ X_k     (two matmuls, PSUM-accumulated)
where D_k is the in-chunk lower-tri block and O_k the previous-chunk block of L.
No carry chain, no serial dependency — every chunk is independent. w_t rounds to
1.0f for t >= 216, so D is shared for chunks >= 2 and O for chunks >= 3; only 6
distinct 128x128 matrices total, uploaded once.

Precision/traffic: the harness gate is rel_err < 2e-2; bf16 I/O (x, W, y all
bf16, PE accumulates in f32 PSUM) gives ~2.7e-3 rel err and HALVES the HBM
traffic vs f32: 8 MiB in + 8 MiB out per core, against a ~358 GB/s per-core
HBM limit -> ~47 us floor (the f32 baseline's floor was ~104 us).

Sharding: pure data parallelism — 4 of the 32 batches per core, no comms.
Host converts f32->bf16 on the way in and bf16->f32 on the way out.

Schedule per core: loads grouped 8 chunks (512 KiB) per DMA on the SP HWDGE
ring, stores on the ACT ring; PSUM->SBUF downcast copies alternate between the
ACT and DVE engines so neither becomes the bottleneck.
"""

import os
import sys

import numpy as np

for _p in ("/opt/trn_rl_repo",):
    if os.path.isdir(_p) and _p not in sys.path:
        sys.path.append(_p)

import ml_dtypes

import concourse.bass as bass
import concourse.mybir as mybir
from concourse.bass_utils import run_bass_kernel_spmd
from concourse.tile import TileContext
from concourse.vector_clock import ScopedClock

# ---------------------------------------------------------------------------
# Workaround: TileContext's tail drain puts every owed proc's sem wait on one
# Drain instruction; walrus codegen allows only one sync wait per instruction,
# so any kernel touching more than a few procs fails codegen with "Too many
# sync wait commands". Split the waits across SP nops, one wait each.
# ---------------------------------------------------------------------------
_MAX_WAITS = 1


def _split_drain_and_barrier(self, tick_clock, wait_clock):
    carrier = self.nc.sync.nop(nofuse=True, hint="drain_wait_carrier")
    wait_clock.add_sem_waits(
        carrier.ins, ScopedClock({None: tick_clock.global_clock})
    )
    si = carrier.ins.sync_info
    if si is not None and len(si.on_wait) > _MAX_WAITS:
        waits = list(si.on_wait)
        carrier.ins.sync_info = mybir.SyncInfo(
            on_wait=waits[:_MAX_WAITS], on_update=list(si.on_update)
        )
        rest = waits[_MAX_WAITS:]
        for i in range(0, len(rest), _MAX_WAITS):
            nop = self.nc.sync.nop(nofuse=True, hint="drain_wait_spill")
            nop.ins.sync_info = mybir.SyncInfo(
                on_wait=rest[i : i + _MAX_WAITS], on_update=[]
            )
    self.nc.sync.drain()

    self.nc.all_engine_barrier()
    assert self.sems is not None
    popped = self.nc._tile_sem_poison_stack.pop()
    assert popped is self._sem_poison
    self.nc.clear_and_free_semaphores(list(self.sems.allocated().values()))
    self.nc.all_engine_barrier()


TileContext._drain_and_barrier = _split_drain_and_barrier

# ---------------------------------------------------------------------------
# Same walrus limitation for regular instructions: Tile attaches up to ~4 sem
# waits to one instruction; this walrus rejects more than WAIT_CAPS[type] sync
# wait commands per instruction. Spill the extras onto same-engine NoOps
# inserted right before the instruction (engines execute their stream in BB
# order, so the waits still complete before the instruction runs).
# ---------------------------------------------------------------------------

_WAIT_CAP_DEFAULT = 1
_WAIT_CAPS = {
    "InstEventSemaphore": 2,
}
_spill_counter = [0]


def spill_excess_waits(nc):
    for fn in nc.m.functions:
        for bb in fn.blocks:
            insts = bb.instructions
            i = 0
            while i < len(insts):
                inst = insts[i]
                si = inst.sync_info
                if si is None or not si.on_wait:
                    i += 1
                    continue
                cap = _WAIT_CAPS.get(type(inst).__name__, _WAIT_CAP_DEFAULT)
                waits = list(si.on_wait)
                if len(waits) <= cap:
                    i += 1
                    continue
                keep = waits[-cap:]
                rest = waits[:-cap]
                inst.sync_info = mybir.SyncInfo(
                    on_wait=keep, on_update=list(si.on_update)
                )
                carriers = []
                for j in range(0, len(rest), _WAIT_CAP_DEFAULT):
                    _spill_counter[0] += 1
                    nop = mybir.InstNoOp(name=f"spillw-{_spill_counter[0]}")
                    nop.engine = inst.engine
                    nop.sync_info = mybir.SyncInfo(
                        on_wait=rest[j : j + _WAIT_CAP_DEFAULT], on_update=[]
                    )
                    carriers.append(nop)
                for off, nop in enumerate(carriers):
                    insts.insert(i + off, nop)
                i += len(carriers) + 1


B, T, F = 32, 4096, 256
NCORES = 8
BL = B // NCORES  # local batches per core
C = 128  # time chunk
NCHUNK = T // C
NMAT = 6  # D0, D1, Dc, O1, O2, Oc


def _coeffs():
    """The 6 distinct 128x128 L-blocks, as lhsT ([t_in, t_out]) bf16."""
    alpha = np.float64(np.float32(2.0 / 26.0))
    oma = np.float64(np.float32(1.0 - 2.0 / 26.0))
    t = np.arange(1, T, dtype=np.float32)
    w32 = np.maximum(
        np.float32(1.0) - np.float32(oma) ** (t + np.float32(1.0)),
        np.float32(1e-10),
    ).astype(np.float32)
    a = np.ones(T, dtype=np.float64)
    b = np.zeros(T, dtype=np.float64)
    a[1:] = oma / w32.astype(np.float64)
    b[1:] = alpha / w32.astype(np.float64)
    b[0] = 1.0

    def build_pair(k):
        lo = k * C
        base = max(0, lo - C)
        P = np.cumprod(a[base : lo + C])

        def L(tt, ss):
            return b[ss] * (P[tt - base] / P[ss - base])

        D = np.zeros((C, C))
        O = np.zeros((C, C))
        for tt in range(C):
            for ss in range(tt + 1):
                D[tt, ss] = L(lo + tt, lo + ss)
            if k > 0:
                for ss in range(C):
                    O[tt, ss] = L(lo + tt, lo - C + ss)
        return D, O

    d0, _ = build_pair(0)
    d1, o1 = build_pair(1)
    dc, o2 = build_pair(2)
    _, oc = build_pair(3)
    # lhsT layout per matrix: [t_in (partition), t_out]; stack -> (128, 6, 128)
    wt = np.stack(
        [m.T for m in (d0, d1, dc, o1, o2, oc)], axis=0
    ).astype(np.float32)
    wt = np.ascontiguousarray(wt.transpose(1, 0, 2)).astype(ml_dtypes.bfloat16)
    return wt


_WT = _coeffs()
# matrix indices in _WT
D0, D1, DC, O1, O2, OC = range(6)


def _sel(k):
    """(off_idx or None, diag_idx) for chunk k."""
    if k == 0:
        return None, D0
    if k == 1:
        return O1, D1
    if k == 2:
        return O2, DC
    return OC, DC


GROUP = 8  # chunks per DMA group (512 KiB bf16 loads)


def build_nc(repeats=1, variant="full", xbufs=8, ebufs=8, spill=True,
             bench_io=False, group=GROUP):
    f32 = mybir.dt.float32
    bf16 = mybir.dt.bfloat16
    nc = bass.Bass(trn_type="TRN2")
    if bench_io:
        # Timing-only NEFF: tiny external I/O (dispatch payload over axon is
        # per-call, ~100ms for the real 384MB), real traffic hits internal
        # DRAM scratch instead. Data is garbage; timing is identical.
        xin = nc.dram_tensor("x", [1, 4], bf16, kind="ExternalInput")
        wt = nc.dram_tensor("wt", [128, NMAT, C], bf16, kind="ExternalInput")
        yout = nc.dram_tensor("y", [1, 4], bf16, kind="ExternalOutput")
        x = nc.dram_tensor("xscratch", [BL, T, F], bf16)
        y = nc.dram_tensor("yscratch", [BL, T, F], bf16)
    else:
        x = nc.dram_tensor("x", [BL, T, F], bf16, kind="ExternalInput")
        wt = nc.dram_tensor("wt", [128, NMAT, C], bf16, kind="ExternalInput")
        y = nc.dram_tensor("y", [BL, T, F], bf16, kind="ExternalOutput")

    with TileContext(nc) as tc:
        with (
            tc.tile_pool(name="wpool", bufs=1) as wpool,
            tc.tile_pool(name="xpool", bufs=xbufs) as xpool,
            tc.tile_pool(name="epool", bufs=ebufs) as epool,
            tc.tile_pool(name="psum", bufs=8, space="PSUM") as ppool,
        ):
            w_tile = wpool.tile([128, NMAT, C], bf16)
            nc.sync.dma_start(out=w_tile[:], in_=wt[:])
            if bench_io:
                iot = wpool.tile([1, 4], bf16, name="iot")
                nc.sync.dma_start(out=iot[:], in_=xin[:])
                nc.sync.dma_start(out=yout[:], in_=iot[:])
            for _rep in range(repeats):
                _emit_pass(nc, tc, x, y, w_tile,
                           (xpool, epool, ppool), variant, group=group)
    if spill:
        spill_excess_waits(nc)
    return nc


def _emit_pass(nc, tc, x, y, w_tile, pools, variant="full", group=GROUP):
    xpool, epool, ppool = pools
    f32 = mybir.dt.float32
    bf16 = mybir.dt.bfloat16

    xr = x.rearrange("b (g t) f -> b t g f", t=C)
    yr = y.rearrange("b (g t) f -> b t g f", t=C)

    if variant.startswith("dmabig"):
        # pure-DMA bandwidth probe with `nch` chunks per DMA
        nch = int(variant[6:])
        for g0 in range(0, NCHUNK, nch):
            for b in range(BL):
                xt = xpool.tile([C, nch, F], bf16, tag="xtb")
                nc.sync.dma_start(out=xt[:], in_=xr[b, :, g0 : g0 + nch, :])
                nc.scalar.dma_start(out=yr[b, :, g0 : g0 + nch, :], in_=xt[:])
        return

    prev = [None] * BL  # (tile, j) holding chunk k-1 per batch
    for g0 in range(0, NCHUNK, group):
        xts, ets = [], []
        for b in range(BL):
            xt = xpool.tile([C, group, F], bf16, tag="xt")
            nc.sync.dma_start(out=xt[:], in_=xr[b, :, g0 : g0 + group, :])
            xts.append(xt)
            ets.append(
                epool.tile([C, group, F], bf16, tag="et", name=f"et_{g0}_{b}")
            )
        for j in range(group):
            k = g0 + j
            osel, dsel = _sel(k)
            for b in range(BL):
                pt = ppool.tile([C, F], f32, tag="pt")
                if osel is None:
                    nc.tensor.matmul(
                        pt[:], w_tile[:, dsel, :], xts[b][:, j, :],
                        start=True, stop=True,
                    )
                else:
                    pxt, pj = (xts[b], j - 1) if j > 0 else prev[b]
                    nc.tensor.matmul(
                        pt[:], w_tile[:, osel, :], pxt[:, pj, :],
                        start=True, stop=False,
                    )
                    nc.tensor.matmul(
                        pt[:], w_tile[:, dsel, :], xts[b][:, j, :],
                        start=False, stop=True,
                    )
                # PSUM f32 -> SBUF bf16 downcast, split by batch so each et
                # tile is written by exactly ONE engine: even b on ACT (same
                # engine as the store dma_start -> structural FIFO ordering),
                # odd b on DVE (store then carries exactly one cross-engine
                # sem wait, within the 1-wait codegen cap — no spill NoOp on
                # the store path).
                if b % 2 == 0:
                    nc.scalar.copy(out=ets[b][:, j, :], in_=pt[:])
                else:
                    nc.vector.tensor_copy(ets[b][:, j, :], pt[:])
        for b in range(BL):
            prev[b] = (xts[b], group - 1)
            nc.scalar.dma_start(
                out=yr[b, :, g0 : g0 + group, :], in_=ets[b][:]
            )


_NC = None


def get_nc():
    global _NC
    if _NC is None:
        _NC = build_nc()
    return _NC


def kernel(x):
    x = np.asarray(x)
    assert x.shape == (B, T, F), x.shape
    xb = np.ascontiguousarray(x.astype(ml_dtypes.bfloat16))
    nc = get_nc()
    in_maps = [
        {"x": xb[c * BL : (c + 1) * BL], "wt": _WT} for c in range(NCORES)
    ]
    res = run_bass_kernel_spmd(nc, in_maps, core_ids=list(range(NCORES)))
    out = np.concatenate(
        [res.results[c]["y"].astype(np.float32) for c in range(NCORES)], axis=0
    )
    return out


# revision 6
# speedup vs baseline: 1.8131x; 1.5974x over previous
"""EMA (ExponentialMovingAverage, adjust=True) over (32, 4096, 256) f32 on 8 trn2 cores.

Math: the reference recurrence is
    e_0 = x_0;  e_t = (alpha*x_t + oma*e_{t-1}) / w_t,  w_t = max(1-oma^(t+1), 1e-10)
i.e. e_t = a_t*e_{t-1} + b_t*x_t with a_t = oma/w_t, b_t = alpha/w_t. Coefficients
depend only on t, so the whole scan is E = L @ X with lower-triangular
L[t,s] = b_s * prod_{u=s+1..t} a_u. L decays like oma^(t-s) (oma=0.923), so it is
effectively banded: truncating at lag >= 129 changes the result by ~3e-7 rel.

Kernel: chunk time into C=128 blocks. For chunk k:
    E_k = O_k @ X_{k-1} + D_k @ X_k     (two matmuls, PSUM-accumulated)
where D_k is the in-chunk lower-tri block and O_k the previous-chunk block of L.
No carry chain, no serial dependency — every chunk is independent. w_t rounds to
1.0f for t >= 216, so D is shared for chunks >= 2 and O for chunks >= 3; only 6
distinct 128x128 matrices total, uploaded once.

Precision/traffic: the harness gate is rel_err < 2e-2; bf16 I/O (x, W, y all
bf16, PE accumulates in f32 PSUM) gives ~2.7e-3 rel err and HALVES the HBM
traffic vs f32: 8 MiB in + 8 MiB out per core, against a ~358 GB/s per-core
HBM limit -> ~47 us floor (the f32 baseline's floor was ~104 us).

Sharding: pure data parallelism — 4 of the 32 batches per core, no comms.
Host converts f32->bf16 on the way in and bf16->f32 on the way out.

Schedule per core: loads grouped 8 chunks (512 KiB) per DMA on the SP HWDGE
ring, stores on the ACT ring; PSUM->SBUF downcast copies alternate between the
ACT and DVE engines so neither becomes the bottleneck.
"""

import os
import sys

import numpy as np

for _p in ("/opt/trn_rl_repo",):
    if os.path.isdir(_p) and _p not in sys.path:
        sys.path.append(_p)

import ml_dtypes

import concourse.bass as bass
import concourse.mybir as mybir
from concourse.bass_utils import run_bass_kernel_spmd
from concourse.tile import TileContext
from concourse.vector_clock import ScopedClock

# ---------------------------------------------------------------------------
# Workaround: TileContext's tail drain puts every owed proc's sem wait on one
# Drain instruction; walrus codegen allows only one sync wait per instruction,
# so any kernel touching more than a few procs fails codegen with "Too many
# sync wait commands". Split the waits across SP nops, one wait each.
# ---------------------------------------------------------------------------
_MAX_WAITS = 1


def _split_drain_and_barrier(self, tick_clock, wait_clock):
    carrier = self.nc.sync.nop(nofuse=True, hint="drain_wait_carrier")
    wait_clock.add_sem_waits(
        carrier.ins, ScopedClock({None: tick_clock.global_clock})
    )
    si = carrier.ins.sync_info
    if si is not None and len(si.on_wait) > _MAX_WAITS:
        waits = list(si.on_wait)
        carrier.ins.sync_info = mybir.SyncInfo(
            on_wait=waits[:_MAX_WAITS], on_update=list(si.on_update)
        )
        rest = waits[_MAX_WAITS:]
        for i in range(0, len(rest), _MAX_WAITS):
            nop = self.nc.sync.nop(nofuse=True, hint="drain_wait_spill")
            nop.ins.sync_info = mybir.SyncInfo(
                on_wait=rest[i : i + _MAX_WAITS], on_update=[]
            )
    self.nc.sync.drain()

    self.nc.all_engine_barrier()
    assert self.sems is not None
    popped = self.nc._tile_sem_poison_stack.pop()
    assert popped is self._sem_poison
    self.nc.clear_and_free_semaphores(list(self.sems.allocated().values()))
    self.nc.all_engine_barrier()


TileContext._drain_and_barrier = _split_drain_and_barrier

# ---------------------------------------------------------------------------
# Same walrus limitation for regular instructions: Tile attaches up to ~4 sem
# waits to one instruction; this walrus rejects more than WAIT_CAPS[type] sync
# wait commands per instruction. Spill the extras onto same-engine NoOps
# inserted right before the instruction (engines execute their stream in BB
# order, so the waits still complete before the instruction runs).
# ---------------------------------------------------------------------------

_WAIT_CAP_DEFAULT = 1
_WAIT_CAPS = {
    "InstEventSemaphore": 2,
}
_spill_counter = [0]


def spill_excess_waits(nc):
    for fn in nc.m.functions:
        for bb in fn.blocks:
            insts = bb.instructions
            i = 0
            while i < len(insts):
                inst = insts[i]
                si = inst.sync_info
                if si is None or not si.on_wait:
                    i += 1
                    continue
                cap = _WAIT_CAPS.get(type(inst).__name__, _WAIT_CAP_DEFAULT)
                waits = list(si.on_wait)
                if len(waits) <= cap:
                    i += 1
                    continue
                keep = waits[-cap:]
                rest = waits[:-cap]
                inst.sync_info = mybir.SyncInfo(
                    on_wait=keep, on_update=list(si.on_update)
                )
                carriers = []
                for j in range(0, len(rest), _WAIT_CAP_DEFAULT):
                    _spill_counter[0] += 1
                    nop = mybir.InstNoOp(name=f"spillw-{_spill_counter[0]}")
                    nop.engine = inst.engine
                    nop.sync_info = mybir.SyncInfo(
                        on_wait=rest[j : j + _WAIT_CAP_DEFAULT], on_update=[]
                    )
                    carriers.append(nop)
                for off, nop in enumerate(carriers):
                    insts.insert(i + off, nop)
                i += len(carriers) + 1


B, T, F = 32, 4096, 256
NCORES = 8
BL = B // NCORES  # local batches per core
C = 128  # time chunk
NCHUNK = T // C
NMAT = 6  # D0, D1, Dc, O1, O2, Oc


def _coeffs():
    """The 6 distinct 128x128 L-blocks, as lhsT ([t_in, t_out]) bf16."""
    alpha = np.float64(np.float32(2.0 / 26.0))
    oma = np.float64(np.float32(1.0 - 2.0 / 26.0))
    t = np.arange(1, T, dtype=np.float32)
    w32 = np.maximum(
        np.float32(1.0) - np.float32(oma) ** (t + np.float32(1.0)),
        np.float32(1e-10),
    ).astype(np.float32)
    a = np.ones(T, dtype=np.float64)
    b = np.zeros(T, dtype=np.float64)
    a[1:] = oma / w32.astype(np.float64)
    b[1:] = alpha / w32.astype(np.float64)
    b[0] = 1.0

    def build_pair(k):
        lo = k * C
        base = max(0, lo - C)
        P = np.cumprod(a[base : lo + C])

        def L(tt, ss):
            return b[ss] * (P[tt - base] / P[ss - base])

        D = np.zeros((C, C))
        O = np.zeros((C, C))
        for tt in range(C):
            for ss in range(tt + 1):
                D[tt, ss] = L(lo + tt, lo + ss)
            if k > 0:
                for ss in range(C):
                    O[tt, ss] = L(lo + tt, lo - C + ss)
        return D, O

    d0, _ = build_pair(0)
    d1, o1 = build_pair(1)
    dc, o2 = build_pair(2)
    _, oc = build_pair(3)
    # lhsT layout per matrix: [t_in (partition), t_out]; stack -> (128, 6, 128)
    wt = np.stack(
        [m.T for m in (d0, d1, dc, o1, o2, oc)], axis=0
    ).astype(np.float32)
    wt = np.ascontiguousarray(wt.transpose(1, 0, 2)).astype(ml_dtypes.bfloat16)
    return wt


_WT = _coeffs()
# matrix indices in _WT
D0, D1, DC, O1, O2, OC = range(6)


def _sel(k):
    """(off_idx or None, diag_idx) for chunk k."""
    if k == 0:
        return None, D0
    if k == 1:
        return O1, D1
    if k == 2:
        return O2, DC
    return OC, DC


GROUP = 8  # chunks per DMA group
NG = NCHUNK // GROUP  # DMA groups per pass

# DRAM layout (host-permuted): [NG, C, GROUP, BL, F] — exactly the SBUF tile
# order, so every group load/store is ONE fully-contiguous 2 MiB DMA (16 KiB
# per partition line; ~99% descriptor efficiency vs 76% at the natural
# [B, T, F] layout's 512 B pieces).
XSHAPE = [NG, C, GROUP, BL, F]


def build_nc(repeats=1, variant="full", xbufs=4, ebufs=3, spill=True,
             bench_io=False):
    f32 = mybir.dt.float32
    bf16 = mybir.dt.bfloat16
    nc = bass.Bass(trn_type="TRN2")
    if bench_io:
        # Timing-only NEFF: tiny external I/O (dispatch payload over axon is
        # per-call, ~100ms for the real 384MB), real traffic hits internal
        # DRAM scratch instead. Data is garbage; timing is identical.
        xin = nc.dram_tensor("x", [1, 4], bf16, kind="ExternalInput")
        wt = nc.dram_tensor("wt", [128, NMAT, C], bf16, kind="ExternalInput")
        yout = nc.dram_tensor("y", [1, 4], bf16, kind="ExternalOutput")
        x = nc.dram_tensor("xscratch", XSHAPE, bf16)
        y = nc.dram_tensor("yscratch", XSHAPE, bf16)
    else:
        x = nc.dram_tensor("x", XSHAPE, bf16, kind="ExternalInput")
        wt = nc.dram_tensor("wt", [128, NMAT, C], bf16, kind="ExternalInput")
        y = nc.dram_tensor("y", XSHAPE, bf16, kind="ExternalOutput")

    with TileContext(nc) as tc:
        with (
            tc.tile_pool(name="wpool", bufs=1) as wpool,
            tc.tile_pool(name="xpool", bufs=xbufs) as xpool,
            tc.tile_pool(name="epool", bufs=ebufs) as epool,
            tc.tile_pool(name="psum", bufs=6, space="PSUM") as ppool,
        ):
            w_tile = wpool.tile([128, NMAT, C], bf16)
            nc.sync.dma_start(out=w_tile[:], in_=wt[:])
            if bench_io:
                iot = wpool.tile([1, 4], bf16, name="iot")
                nc.sync.dma_start(out=iot[:], in_=xin[:])
                nc.sync.dma_start(out=yout[:], in_=iot[:])
            for _rep in range(repeats):
                _emit_pass(nc, tc, x, y, w_tile, (xpool, epool, ppool),
                           variant)
    if spill:
        spill_excess_waits(nc)
    return nc


def _emit_pass(nc, tc, x, y, w_tile, pools, variant="full"):
    xpool, epool, ppool = pools
    f32 = mybir.dt.float32
    bf16 = mybir.dt.bfloat16

    if variant == "dmaonly":
        # pure-DMA bandwidth probe: loads on SP ring, stores on ACT ring
        for gg in range(NG):
            xt = xpool.tile([C, GROUP, BL, F], bf16, tag="xt")
            nc.sync.dma_start(out=xt[:], in_=x[gg])
            nc.scalar.dma_start(out=y[gg], in_=xt[:])
        return

    prev = None  # x tile of the previous group (chunk k-1 for j == 0)
    for gg in range(NG):
        xt = xpool.tile([C, GROUP, BL, F], bf16, tag="xt")
        nc.sync.dma_start(out=xt[:], in_=x[gg])
        et = epool.tile([C, GROUP, BL, F], bf16, tag="et", name=f"et_{gg}")
        for j in range(GROUP):
            k = gg * GROUP + j
            osel, dsel = _sel(k)
            for p in range(BL // 2):  # batch pairs -> N=512 matmuls
                b0 = 2 * p
                pt = ppool.tile([C, 2 * F], f32, tag="pt")
                rhs = xt[:, j, b0 : b0 + 2, :]
                if osel is None:
                    nc.tensor.matmul(
                        pt[:], w_tile[:, dsel, :], rhs, start=True, stop=True
                    )
                else:
                    prhs = (xt if j > 0 else prev)[
                        :, j - 1 if j > 0 else GROUP - 1, b0 : b0 + 2, :
                    ]
                    nc.tensor.matmul(
                        pt[:], w_tile[:, osel, :], prhs, start=True, stop=False
                    )
                    nc.tensor.matmul(
                        pt[:], w_tile[:, dsel, :], rhs, start=False, stop=True
                    )
                # PSUM f32 -> SBUF bf16 downcast, split by batch-pair so et is
                # written by exactly two engines: pair 0 on ACT (same engine
                # as the store dma_start -> structural FIFO ordering), pair 1
                # on DVE (store then carries exactly one cross-engine sem
                # wait, within the 1-wait codegen cap — no spill NoOp on the
                # store path).
                if p == 0:
                    nc.scalar.copy(out=et[:, j, b0 : b0 + 2, :], in_=pt[:])
                else:
                    nc.vector.tensor_copy(et[:, j, b0 : b0 + 2, :], pt[:])
        nc.scalar.dma_start(out=y[gg], in_=et[:])
        prev = xt


_NC = None


def get_nc():
    global _NC
    if _NC is None:
        _NC = build_nc()
    return _NC


def _pack(xc):
    """[BL, T, F] -> device layout [NG, C, GROUP, BL, F]."""
    v = xc.reshape(BL, NG, GROUP, C, F)
    return np.ascontiguousarray(v.transpose(1, 3, 2, 0, 4))


def _unpack(yh):
    """device layout [NG, C, GROUP, BL, F] -> [BL, T, F]."""
    return yh.transpose(3, 0, 2, 1, 4).reshape(BL, T, F)


def kernel(x):
    x = np.asarray(x)
    assert x.shape == (B, T, F), x.shape
    xb = x.astype(ml_dtypes.bfloat16)
    nc = get_nc()
    in_maps = [
        {"x": _pack(xb[c * BL : (c + 1) * BL]), "wt": _WT}
        for c in range(NCORES)
    ]
    res = run_bass_kernel_spmd(nc, in_maps, core_ids=list(range(NCORES)))
    out = np.concatenate(
        [
            _unpack(res.results[c]["y"]).astype(np.float32)
            for c in range(NCORES)
        ],
        axis=0,
    )
    return out
